# revision 19
# baseline (speedup 1.0000x reference)
"""Causal self-attention with RoPE, sharded over 8 TRN2 NeuronCores.

Sharding: data-parallel over B (4 ways) x tensor-parallel over heads
(2 ways, 6 heads each). Each core computes qkv projection, RoPE,
causal attention and a partial output projection for its (batch,
head-half); the host sums the two head-half partials per batch.

Device matmuls run in bf16 (f32 PSUM accumulate). All attention
matmuls are uniform (K=128, M=128) shapes: kT is zero-padded per head
to a full 128-partition stationary, and V carries a ones column (for
the softmax denominator) plus zero padding to 128 columns. Softmax
needs no max-subtraction at these score magnitudes.

RoPE pairs are host-permuted into contiguous even/odd halves per head
(legal because the QK contraction is invariant to permuting head dims
as long as q and k share the permutation, and V is not roped).
"""

import numpy as np
import ml_dtypes

import concourse.bass as bass
import concourse.tile as tile
import concourse.mybir as mybir
from concourse import bacc
from concourse.bass_utils import run_bass_kernel_spmd

B, T, C, H, D = 4, 2048, 768, 12, 64
HL = H // 2          # heads per core
TB = T // 128        # 16 t-blocks
CB = C // 128        # 6 contraction blocks
NCORES = 8

F32 = mybir.dt.float32
BF16 = mybir.dt.bfloat16
AF = mybir.ActivationFunctionType

_CACHED_NC = None
DEBUG_DUMPS = False


def build_nc():
    nc = bacc.Bacc("TRN2", target_bir_lowering=False)

    xT = nc.declare_dram_parameter("xT", [C, T], BF16, isOutput=False)
    wqkvT = nc.declare_dram_parameter("wqkvT", [C, 3 * HL * D], BF16, isOutput=False)
    wpT = nc.declare_dram_parameter("wpT", [HL * D, C], BF16, isOutput=False)
    sinr = nc.declare_dram_parameter("sinr", [T, HL * D], BF16, isOutput=False)
    cosr = nc.declare_dram_parameter("cosr", [T, HL * D], BF16, isOutput=False)
    tri = nc.declare_dram_parameter("tri", [128, 128], BF16, isOutput=False)
    negi = nc.declare_dram_parameter("negi", [128, 128], BF16, isOutput=False)
    ident = nc.declare_dram_parameter("ident", [128, 128], BF16, isOutput=False)
    out = nc.declare_dram_parameter("out", [T, C], F32, isOutput=True)
    dbg = {}
    if DEBUG_DUMPS:
        dbg["dq"] = nc.declare_dram_parameter("dq", [128, 3 * T], BF16, isOutput=True)
        dbg["dk"] = nc.declare_dram_parameter("dk", [128, HL * T], BF16, isOutput=True)
        dbg["dv"] = nc.declare_dram_parameter("dv", [128, HL * 128], BF16, isOutput=True)
        dbg["dy"] = nc.declare_dram_parameter("dy", [64, T], BF16, isOutput=True)
        dbg["dden"] = nc.declare_dram_parameter("dden", [64, 4 * 512], F32, isOutput=True)

    with tile.TileContext(nc) as tc:
        with (
            tc.tile_pool(name="persist", bufs=1) as persist,
            tc.tile_pool(name="pPp", bufs=4) as pPp,
            tc.tile_pool(name="small", bufs=3) as small,
        ):
            # ---- persistent SBUF tensors ----
            # qT_all: head-pair block hb occupies cols [hb*2048, (hb+1)*2048);
            # head h lives at rows (h%2)*64 of block h//2.
            qT_all = persist.tile([128, 3 * T], BF16, tag="qT", name="qT_all")
            # kTp_all: head h occupies cols [h*2048, ...), rows (h%2)*64,
            # other 64 rows are zeros (full-K stationary for uniform matmuls).
            kTp_all = persist.tile([128, HL * T], BF16, tag="kT", name="kTp_all")
            # v tiles: per t-block, head h at cols [h*128, h*128+64] = v,
            # col h*128+64 = ones, cols h*128+65.. = zeros.
            v_sb = [persist.tile([128, HL, 128], BF16, tag=f"v{i}", name=f"v{i}")
                    for i in range(TB)]
            wp_sb = [persist.tile([128, C], BF16, tag=f"wp{i}", name=f"wp{i}")
                     for i in range(3)]
            yT64 = [persist.tile([64, T], BF16, tag=f"y64_{i}", name=f"y64_{i}")
                    for i in range(HL)]
            yT128 = [persist.tile([128, T], BF16, tag=f"y128_{i}", name=f"y128_{i}")
                     for i in range(3)]
            tri_sb = persist.tile([128, 128], BF16, tag="tri")
            negi_sb = persist.tile([128, 128], BF16, tag="negi")
            id_sb = persist.tile([128, 128], BF16, tag="ident")

            nc.gpsimd.dma_start(out=tri_sb, in_=tri[:, :])
            nc.gpsimd.dma_start(out=negi_sb, in_=negi[:, :])
            nc.gpsimd.dma_start(out=id_sb, in_=ident[:, :])
            for i in range(3):
                nc.gpsimd.dma_start(out=wp_sb[i], in_=wpT[i * 128:(i + 1) * 128, :])

            # zero the off-head rows of kTp_all once
            kTp_v = kTp_all.rearrange("p (c two t) -> p c two t", c=3, two=2)
            nc.vector.memset(kTp_v[0:64, :, 1:2, :], 0.0)
            nc.vector.memset(kTp_v[64:128, :, 0:1, :], 0.0)
            # zero/ones padding of v tiles once
            for i in range(TB):
                nc.vector.memset(v_sb[i][:, :, D:D + 1], 1.0)
                nc.vector.memset(v_sb[i][:, :, D + 1:128], 0.0)

            # ================= phase 1: qkv + rope + transpose =================
            with (
                tc.tile_pool(name="p1in", bufs=1) as p1in,
                tc.tile_pool(name="p1work", bufs=3) as p1w,
                tc.tile_pool(name="p1psum", bufs=3, space="PSUM") as p1ps,
                tc.tile_pool(name="ptpsum", bufs=2, space="PSUM") as ptps,
            ):
                xT_sb = [p1in.tile([128, T], BF16, tag=f"xT{i}", name=f"xTs{i}")
                         for i in range(CB)]
                w_sb = [p1in.tile([128, 3 * HL * D], BF16, tag=f"w{i}", name=f"ws{i}")
                        for i in range(CB)]
                for i in range(CB):
                    nc.sync.dma_start(out=w_sb[i],
                                      in_=wqkvT[i * 128:(i + 1) * 128, :])
                    for hcol in range(2):
                        nc.sync.dma_start(
                            out=xT_sb[i][:, hcol * 1024:(hcol + 1) * 1024],
                            in_=xT[i * 128:(i + 1) * 128,
                                   hcol * 1024:(hcol + 1) * 1024])

                for tb in range(TB):
                    tsl = bass.ts(tb, 128)
                    ps_qkv = []
                    for chunk in range(3):
                        ps = p1ps.tile([128, 384], F32, tag="qkv", name="psqkv")
                        for cb in range(CB):
                            nc.tensor.matmul(
                                ps,
                                lhsT=xT_sb[cb][:, tsl],
                                rhs=w_sb[cb][:, chunk * 384:(chunk + 1) * 384],
                                start=(cb == 0),
                                stop=(cb == CB - 1),
                            )
                        ps_qkv.append(ps)

                    sin_sb = p1w.tile([128, HL * D], BF16, tag="sin", name="sin_sb")
                    cos_sb = p1w.tile([128, HL * D], BF16, tag="cos", name="cos_sb")
                    nc.gpsimd.dma_start(out=sin_sb, in_=sinr[tsl, :])
                    nc.gpsimd.dma_start(out=cos_sb, in_=cosr[tsl, :])

                    # v: pack t-major (ones/zero padding already set)
                    nc.scalar.copy(
                        v_sb[tb][:, :, 0:D],
                        ps_qkv[2].rearrange("p (h d) -> p h d", h=HL),
                    )

                    # rope directly from PSUM; halves layout (host-permuted):
                    # per head cols [evens(32) | odds(32)]
                    for qk in range(2):
                        ro = p1w.tile([128, HL * D], BF16, tag=f"ro{qk}",
                                      name=f"ro{qk}")
                        t1 = p1w.tile([128, HL * D], BF16, tag="t1", name="t1")
                        t2 = p1w.tile([128, HL * D], BF16, tag="t2", name="t2")
                        nc.vector.tensor_mul(t1, ps_qkv[qk], cos_sb)
                        nc.vector.tensor_mul(t2, ps_qkv[qk], sin_sb)
                        rv = ro.rearrange("p (h half i) -> p h half i", h=HL, half=2)
                        t1v = t1.rearrange("p (h half i) -> p h half i", h=HL, half=2)
                        t2v = t2.rearrange("p (h half i) -> p h half i", h=HL, half=2)
                        nc.vector.tensor_sub(rv[:, :, 0:1, :], t1v[:, :, 0:1, :],
                                             t2v[:, :, 1:2, :])
                        nc.vector.tensor_add(rv[:, :, 1:2, :], t2v[:, :, 0:1, :],
                                             t1v[:, :, 1:2, :])

                        # transpose the 3 128-col blocks into one PSUM tile
                        pt = ptps.tile([128, 384], BF16, tag="pt", name="pt")
                        for cb2 in range(3):
                            nc.tensor.transpose(
                                pt[:, cb2 * 128:(cb2 + 1) * 128],
                                ro[:, cb2 * 128:(cb2 + 1) * 128], id_sb
                            )
                        if qk == 0:
                            qv = qT_all.rearrange("p (c t) -> p c t", c=3)
                            nc.scalar.copy(qv[:, :, tsl],
                                           pt.rearrange("p (c t) -> p c t", c=3))
                        else:
                            ptv = pt.rearrange("p (c t) -> p c t", c=3)
                            nc.scalar.copy(kTp_v[0:64, :, 0:1, tsl],
                                           ptv[0:64, :, :])
                            nc.scalar.copy(kTp_v[64:128, :, 1:2, tsl],
                                           ptv[64:128, :, :])

            # ================= phase 2: attention =================
            with (
                tc.tile_pool(name="psS", bufs=1, space="PSUM") as psS,
                tc.tile_pool(name="psO", bufs=2, space="PSUM") as psO,
            ):
                for h in range(HL):
                    hb = h // 2
                    for sq in range(4):
                        po = psO.tile([128, 512], F32, tag="po", name="po")
                        njs = 4 * sq + 4
                        jgroups = []
                        g = 0
                        want = 4
                        while g < njs:
                            take = min(want, njs - g)
                            jgroups.append(list(range(g, g + take)))
                            g += take
                            want = 2 if want == 4 else 4
                        for jg in jgroups:
                            if len(jg) > 2:
                                ps3 = psS.tile([128, 2048], F32, tag="ps4",
                                               name="ps4", bufs=1)
                                pP3 = pPp.tile([128, 2048], BF16, tag="pP4",
                                               name="pP4", bufs=2)
                            else:
                                ps3 = psS.tile([128, 1024], F32, tag="ps2",
                                               name="ps2", bufs=1)
                                pP3 = pPp.tile([128, 1024], BF16, tag="pP2",
                                               name="pP2", bufs=2)
                            offs = []
                            for slot, j in enumerate(jg):
                                diag_r = j - 4 * sq
                                col0 = 512 * sq + max(0, diag_r) * 128
                                N = 512 * (sq + 1) - col0
                                o = slot * 512
                                nc.tensor.matmul(
                                    ps3[:, o:o + N],
                                    lhsT=kTp_all[:, h * T + j * 128:
                                                 h * T + (j + 1) * 128],
                                    rhs=qT_all[:, hb * T + col0:hb * T + col0 + N],
                                    start=True, stop=(diag_r < 0),
                                )
                                if diag_r >= 0:
                                    # accumulate -1e9 on the upper triangle so
                                    # exp() zeroes the non-causal entries
                                    nc.tensor.matmul(
                                        ps3[:, o:o + 128],
                                        lhsT=negi_sb,
                                        rhs=tri_sb,
                                        start=False, stop=True,
                                        skip_group_check=True,
                                    )
                                offs.append((o, N, col0 - 512 * sq, diag_r))
                            otot = offs[-1][0] + offs[-1][1]
                            nc.scalar.activation(pP3[:, 0:otot], ps3[:, 0:otot],
                                                 AF.Exp, scale=0.125)
                            for (j, (o2, N, ls, diag_r)) in zip(jg, offs):
                                nc.tensor.matmul(
                                    po[:, ls:ls + N],
                                    lhsT=v_sb[j][:, h, :],
                                    rhs=pP3[:, o2:o2 + N],
                                    start=(j == 0),
                                    stop=(j == njs - 1),
                                )
                        posb = small.tile([D + 1, 512], F32, tag="posb", name="posb")
                        nc.vector.tensor_copy(out=posb, in_=po[0:D + 1, :])
                        rden = small.tile([1, 512], F32, tag="rden", name="rden")
                        nc.vector.tensor_copy(out=rden, in_=po[D:D + 1, :])
                        rec = small.tile([1, 512], F32, tag="rec", name="rec")
                        nc.vector.reciprocal_approx_fast(rec, rden)
                        bc = small.tile([64, 512], F32, tag="bc", name="bc")
                        nc.gpsimd.partition_broadcast(bc, rec)
                        if DEBUG_DUMPS and h == 0:
                            nc.sync.dma_start(out=dbg["dden"][:, sq * 512:(sq + 1) * 512], in_=bc)
                        nc.vector.tensor_mul(
                            yT64[h][:, bass.ts(sq, 512)], posb[0:D, :], bc
                        )
                    nc.sync.dma_start(
                        out=yT128[h // 2][(h % 2) * 64:(h % 2) * 64 + 64, :],
                        in_=yT64[h][:, :],
                    )

            if DEBUG_DUMPS:
                nc.sync.dma_start(out=dbg["dq"][:, :], in_=qT_all)
                nc.sync.dma_start(out=dbg["dk"][:, :], in_=kTp_all)
                nc.sync.dma_start(out=dbg["dv"][:, :], in_=v_sb[0].rearrange("p h d -> p (h d)"))
                nc.sync.dma_start(out=dbg["dy"][:, :], in_=yT64[0])

            # ================= phase 3: output projection (partial) ============
            with (
                tc.tile_pool(name="pjpsum", bufs=3, space="PSUM") as pjps,
                tc.tile_pool(name="pjout", bufs=3) as pjout,
            ):
                for tb in range(TB):
                    tsl = bass.ts(tb, 128)
                    for oc0, ocn in ((0, 512), (512, 256)):
                        pp = pjps.tile([128, 512], F32, tag="pp", name="pp")
                        for fb in range(3):
                            nc.tensor.matmul(
                                pp[:, 0:ocn],
                                lhsT=yT128[fb][:, tsl],
                                rhs=wp_sb[fb][:, oc0:oc0 + ocn],
                                start=(fb == 0),
                                stop=(fb == 2),
                            )
                        osb = pjout.tile([128, 512], F32, tag="osb", name="osb")
                        nc.scalar.copy(osb[:, 0:ocn], pp[:, 0:ocn])
                        nc.sync.dma_start(out=out[tsl, oc0:oc0 + ocn],
                                          in_=osb[:, 0:ocn])

    nc.finalize()
    return nc


def _bf16(a):
    return np.ascontiguousarray(np.asarray(a)).astype(ml_dtypes.bfloat16)


# permutation putting rope pairs into contiguous even/odd halves per head
_PERM64 = np.concatenate([np.arange(0, D, 2), np.arange(1, D, 2)])


def _prep_core(c, x, w_qkv, w_proj, sin_rep, cos_rep, tri_m, ident_m):
    b, hh = c // 2, c % 2
    wq = w_qkv[0 * C + hh * 384: 0 * C + hh * 384 + 384].reshape(HL, D, C)
    wk = w_qkv[1 * C + hh * 384: 1 * C + hh * 384 + 384].reshape(HL, D, C)
    wv = w_qkv[2 * C + hh * 384: 2 * C + hh * 384 + 384]
    wq = wq[:, _PERM64, :].reshape(HL * D, C)
    wk = wk[:, _PERM64, :].reshape(HL * D, C)
    w_local = np.concatenate([wq, wk, wv], 0)       # (1152, 768)
    return {
        "xT": _bf16(x[b].T),
        "wqkvT": _bf16(w_local.T),
        "wpT": _bf16(w_proj[:, hh * 384: hh * 384 + 384].T),
        "sinr": sin_rep,
        "cosr": cos_rep,
        "tri": tri_m,
        "ident": ident_m,
    }


def kernel(x, w_qkv, w_proj, rope_sin, rope_cos, _trace=False):
    global _CACHED_NC
    x = np.asarray(x, dtype=np.float32)
    w_qkv = np.asarray(w_qkv, dtype=np.float32)
    w_proj = np.asarray(w_proj, dtype=np.float32)
    rope_sin = np.asarray(rope_sin, dtype=np.float32)
    rope_cos = np.asarray(rope_cos, dtype=np.float32)

    # (T, 384): per head block [table(32) | table(32)]
    sin_rep = _bf16(np.tile(np.concatenate([rope_sin, rope_sin], 1), (1, HL)))
    cos_rep = _bf16(np.tile(np.concatenate([rope_cos, rope_cos], 1), (1, HL)))
    tri_m = _bf16(np.arange(128)[:, None] > np.arange(128)[None, :])
    negi_m = _bf16(np.eye(128) * -1e9)
    ident_m = _bf16(np.eye(128))

    in_maps = [_prep_core(c, x, w_qkv, w_proj, sin_rep, cos_rep, tri_m, ident_m)
               for c in range(NCORES)]
    for m in in_maps:
        m["negi"] = negi_m

    if _CACHED_NC is None:
        _CACHED_NC = build_nc()
    nc = _CACHED_NC

    res = run_bass_kernel_spmd(nc, in_maps, core_ids=list(range(NCORES)),
                               trace=_trace)
    parts = [res.results[c]["out"] for c in range(NCORES)]
    out = np.stack([parts[2 * b] + parts[2 * b + 1] for b in range(B)], 0)
    if _trace:
        return out.astype(np.float32), res
    return out.astype(np.float32)


# revision 20
# speedup vs baseline: 1.1561x; 1.1561x over previous
"""Causal self-attention with RoPE, sharded over 8 TRN2 NeuronCores.

Sharding: data-parallel over B (4 ways) x tensor-parallel over heads
(2 ways, 6 heads each). Each core computes qkv projection, RoPE,
causal attention and a partial output projection for its (batch,
head-half); the host sums the two head-half partials per batch.

Device matmuls run in bf16 (f32 PSUM accumulate). All attention
matmuls are uniform (K=128, M=128) shapes: kT is zero-padded per head
to a full 128-partition stationary, and V carries a ones column (for
the softmax denominator) plus zero padding to 128 columns. Softmax
needs no max-subtraction at these score magnitudes.

RoPE pairs are host-permuted into contiguous even/odd halves per head
(legal because the QK contraction is invariant to permuting head dims
as long as q and k share the permutation, and V is not roped).
"""

import numpy as np
import ml_dtypes

import concourse.bass as bass
import concourse.tile as tile
import concourse.mybir as mybir
from concourse import bacc
from concourse.bass_utils import run_bass_kernel_spmd

B, T, C, H, D = 4, 2048, 768, 12, 64
HL = H // 2          # heads per core
TB = T // 128        # 16 t-blocks
CB = C // 128        # 6 contraction blocks
NCORES = 8

F32 = mybir.dt.float32
BF16 = mybir.dt.bfloat16
AF = mybir.ActivationFunctionType

_CACHED_NC = None
DEBUG_DUMPS = False


def build_nc():
    nc = bacc.Bacc("TRN2", target_bir_lowering=False)

    xT = nc.declare_dram_parameter("xT", [C, T], BF16, isOutput=False)
    wqkvT = nc.declare_dram_parameter("wqkvT", [C, 3 * HL * D], BF16, isOutput=False)
    wpT = nc.declare_dram_parameter("wpT", [HL * D, C], BF16, isOutput=False)
    sinr = nc.declare_dram_parameter("sinr", [T, HL * D], BF16, isOutput=False)
    cosr = nc.declare_dram_parameter("cosr", [T, HL * D], BF16, isOutput=False)
    tri = nc.declare_dram_parameter("tri", [128, 128], BF16, isOutput=False)
    negi = nc.declare_dram_parameter("negi", [128, 128], BF16, isOutput=False)
    ident = nc.declare_dram_parameter("ident", [128, 128], BF16, isOutput=False)
    out = nc.declare_dram_parameter("out", [T, C], F32, isOutput=True)
    dbg = {}
    if DEBUG_DUMPS:
        dbg["dq"] = nc.declare_dram_parameter("dq", [128, 3 * T], BF16, isOutput=True)
        dbg["dk"] = nc.declare_dram_parameter("dk", [128, HL * T], BF16, isOutput=True)
        dbg["dv"] = nc.declare_dram_parameter("dv", [128, HL * 128], BF16, isOutput=True)
        dbg["dy"] = nc.declare_dram_parameter("dy", [64, T], BF16, isOutput=True)
        dbg["dden"] = nc.declare_dram_parameter("dden", [64, 4 * 512], F32, isOutput=True)

    with tile.TileContext(nc) as tc:
        with (
            tc.tile_pool(name="persist", bufs=1) as persist,
            tc.tile_pool(name="pPp", bufs=6) as pPp,
            tc.tile_pool(name="small", bufs=3) as small,
        ):
            # ---- persistent SBUF tensors ----
            # qT_all: head-pair block hb occupies cols [hb*2048, (hb+1)*2048);
            # head h lives at rows (h%2)*64 of block h//2.
            qT_all = persist.tile([128, 3 * T], BF16, tag="qT", name="qT_all")
            # kTp_all: head h occupies cols [h*2048, ...), rows (h%2)*64,
            # other 64 rows are zeros (full-K stationary for uniform matmuls).
            kTp_all = persist.tile([128, HL * T], BF16, tag="kT", name="kTp_all")
            # v tiles: per t-block, head h at cols [h*128, h*128+64] = v,
            # col h*128+64 = ones, cols h*128+65.. = zeros.
            v_sb = [persist.tile([128, HL, 128], BF16, tag=f"v{i}", name=f"v{i}")
                    for i in range(TB)]
            wp_sb = [persist.tile([128, C], BF16, tag=f"wp{i}", name=f"wp{i}")
                     for i in range(3)]
            yT64 = [persist.tile([64, T], BF16, tag=f"y64_{i}", name=f"y64_{i}")
                    for i in range(HL)]
            yT128 = [persist.tile([128, T], BF16, tag=f"y128_{i}", name=f"y128_{i}")
                     for i in range(3)]
            tri_sb = persist.tile([128, 128], BF16, tag="tri")
            negi_sb = persist.tile([128, 128], BF16, tag="negi")
            id_sb = persist.tile([128, 128], BF16, tag="ident")

            nc.gpsimd.dma_start(out=tri_sb, in_=tri[:, :])
            nc.gpsimd.dma_start(out=negi_sb, in_=negi[:, :])
            nc.gpsimd.dma_start(out=id_sb, in_=ident[:, :])
            for i in range(3):
                nc.gpsimd.dma_start(out=wp_sb[i], in_=wpT[i * 128:(i + 1) * 128, :])

            # zero the off-head rows of kTp_all once
            kTp_v = kTp_all.rearrange("p (c two t) -> p c two t", c=3, two=2)
            nc.vector.memset(kTp_v[0:64, :, 1:2, :], 0.0)
            nc.vector.memset(kTp_v[64:128, :, 0:1, :], 0.0)
            # zero/ones padding of v tiles once
            for i in range(TB):
                nc.vector.memset(v_sb[i][:, :, D:D + 1], 1.0)
                nc.vector.memset(v_sb[i][:, :, D + 1:128], 0.0)

            # ================= phase 1: qkv + rope + transpose =================
            with (
                tc.tile_pool(name="p1in", bufs=1) as p1in,
                tc.tile_pool(name="p1work", bufs=3) as p1w,
                tc.tile_pool(name="p1psum", bufs=3, space="PSUM") as p1ps,
                tc.tile_pool(name="ptpsum", bufs=2, space="PSUM") as ptps,
            ):
                xT_sb = [p1in.tile([128, T], BF16, tag=f"xT{i}", name=f"xTs{i}")
                         for i in range(CB)]
                w_sb = [p1in.tile([128, 3 * HL * D], BF16, tag=f"w{i}", name=f"ws{i}")
                        for i in range(CB)]
                for i in range(CB):
                    nc.sync.dma_start(out=xT_sb[i], in_=xT[i * 128:(i + 1) * 128, :])
                    nc.sync.dma_start(out=w_sb[i], in_=wqkvT[i * 128:(i + 1) * 128, :])

                for tb in range(TB):
                    tsl = bass.ts(tb, 128)
                    ps_qkv = []
                    for chunk in range(3):
                        ps = p1ps.tile([128, 384], F32, tag="qkv", name="psqkv")
                        for cb in range(CB):
                            nc.tensor.matmul(
                                ps,
                                lhsT=xT_sb[cb][:, tsl],
                                rhs=w_sb[cb][:, chunk * 384:(chunk + 1) * 384],
                                start=(cb == 0),
                                stop=(cb == CB - 1),
                            )
                        ps_qkv.append(ps)

                    sin_sb = p1w.tile([128, HL * D], BF16, tag="sin", name="sin_sb")
                    cos_sb = p1w.tile([128, HL * D], BF16, tag="cos", name="cos_sb")
                    nc.gpsimd.dma_start(out=sin_sb, in_=sinr[tsl, :])
                    nc.gpsimd.dma_start(out=cos_sb, in_=cosr[tsl, :])

                    # v: pack t-major (ones/zero padding already set)
                    nc.scalar.copy(
                        v_sb[tb][:, :, 0:D],
                        ps_qkv[2].rearrange("p (h d) -> p h d", h=HL),
                    )

                    # rope directly from PSUM; halves layout (host-permuted):
                    # per head cols [evens(32) | odds(32)]
                    for qk in range(2):
                        ro = p1w.tile([128, HL * D], BF16, tag=f"ro{qk}",
                                      name=f"ro{qk}")
                        t1 = p1w.tile([128, HL * D], BF16, tag="t1", name="t1")
                        t2 = p1w.tile([128, HL * D], BF16, tag="t2", name="t2")
                        nc.vector.tensor_mul(t1, ps_qkv[qk], cos_sb)
                        nc.vector.tensor_mul(t2, ps_qkv[qk], sin_sb)
                        rv = ro.rearrange("p (h half i) -> p h half i", h=HL, half=2)
                        t1v = t1.rearrange("p (h half i) -> p h half i", h=HL, half=2)
                        t2v = t2.rearrange("p (h half i) -> p h half i", h=HL, half=2)
                        nc.vector.tensor_sub(rv[:, :, 0:1, :], t1v[:, :, 0:1, :],
                                             t2v[:, :, 1:2, :])
                        nc.vector.tensor_add(rv[:, :, 1:2, :], t2v[:, :, 0:1, :],
                                             t1v[:, :, 1:2, :])

                        # transpose the 3 128-col blocks into one PSUM tile
                        pt = ptps.tile([128, 384], BF16, tag="pt", name="pt")
                        for cb2 in range(3):
                            nc.tensor.transpose(
                                pt[:, cb2 * 128:(cb2 + 1) * 128],
                                ro[:, cb2 * 128:(cb2 + 1) * 128], id_sb
                            )
                        if qk == 0:
                            qv = qT_all.rearrange("p (c t) -> p c t", c=3)
                            nc.scalar.copy(qv[:, :, tsl],
                                           pt.rearrange("p (c t) -> p c t", c=3))
                        else:
                            ptv = pt.rearrange("p (c t) -> p c t", c=3)
                            nc.scalar.copy(kTp_v[0:64, :, 0:1, tsl],
                                           ptv[0:64, :, :])
                            nc.scalar.copy(kTp_v[64:128, :, 1:2, tsl],
                                           ptv[64:128, :, :])

            # ================= phase 2: attention =================
            with (
                tc.tile_pool(name="psS", bufs=2, space="PSUM") as psS,
                tc.tile_pool(name="psO", bufs=3, space="PSUM") as psO,
            ):
                for h in range(HL):
                    hb = h // 2
                    for sq in range(4):
                        po = psO.tile([128, 512], F32, tag="po", name="po")
                        njs = 4 * sq + 4
                        jgroups = [list(range(g, min(g + 2, njs)))
                                   for g in range(0, njs, 2)]
                        for jg in jgroups:
                            ps3 = psS.tile([128, 1024], F32, tag="ps3", name="ps3")
                            pP3 = pPp.tile([128, 1024], BF16, tag="pP", name="pP")
                            offs = []
                            for slot, j in enumerate(jg):
                                diag_r = j - 4 * sq
                                col0 = 512 * sq + max(0, diag_r) * 128
                                N = 512 * (sq + 1) - col0
                                o = slot * 512
                                nc.tensor.matmul(
                                    ps3[:, o:o + N],
                                    lhsT=kTp_all[:, h * T + j * 128:
                                                 h * T + (j + 1) * 128],
                                    rhs=qT_all[:, hb * T + col0:hb * T + col0 + N],
                                    start=True, stop=(diag_r < 0),
                                )
                                if diag_r >= 0:
                                    # accumulate -1e9 on the upper triangle so
                                    # exp() zeroes the non-causal entries
                                    nc.tensor.matmul(
                                        ps3[:, o:o + 128],
                                        lhsT=negi_sb,
                                        rhs=tri_sb,
                                        start=False, stop=True,
                                        skip_group_check=True,
                                    )
                                offs.append((o, N, col0 - 512 * sq, diag_r))
                            otot = offs[-1][0] + offs[-1][1]
                            nc.scalar.activation(pP3[:, 0:otot], ps3[:, 0:otot],
                                                 AF.Exp, scale=0.125)
                            for (j, (o2, N, ls, diag_r)) in zip(jg, offs):
                                nc.tensor.matmul(
                                    po[:, ls:ls + N],
                                    lhsT=v_sb[j][:, h, :],
                                    rhs=pP3[:, o2:o2 + N],
                                    start=(j == 0),
                                    stop=(j == njs - 1),
                                )
                        posb = small.tile([D + 1, 512], F32, tag="posb", name="posb")
                        nc.vector.tensor_copy(out=posb, in_=po[0:D + 1, :])
                        rden = small.tile([1, 512], F32, tag="rden", name="rden")
                        nc.vector.tensor_copy(out=rden, in_=po[D:D + 1, :])
                        rec = small.tile([1, 512], F32, tag="rec", name="rec")
                        nc.vector.reciprocal_approx_fast(rec, rden)
                        bc = small.tile([64, 512], F32, tag="bc", name="bc")
                        nc.gpsimd.partition_broadcast(bc, rec)
                        if DEBUG_DUMPS and h == 0:
                            nc.sync.dma_start(out=dbg["dden"][:, sq * 512:(sq + 1) * 512], in_=bc)
                        nc.vector.tensor_mul(
                            yT64[h][:, bass.ts(sq, 512)], posb[0:D, :], bc
                        )
                    nc.sync.dma_start(
                        out=yT128[h // 2][(h % 2) * 64:(h % 2) * 64 + 64, :],
                        in_=yT64[h][:, :],
                    )

            if DEBUG_DUMPS:
                nc.sync.dma_start(out=dbg["dq"][:, :], in_=qT_all)
                nc.sync.dma_start(out=dbg["dk"][:, :], in_=kTp_all)
                nc.sync.dma_start(out=dbg["dv"][:, :], in_=v_sb[0].rearrange("p h d -> p (h d)"))
                nc.sync.dma_start(out=dbg["dy"][:, :], in_=yT64[0])

            # ================= phase 3: output projection (partial) ============
            with (
                tc.tile_pool(name="pjpsum", bufs=3, space="PSUM") as pjps,
                tc.tile_pool(name="pjout", bufs=3) as pjout,
            ):
                for tb in range(TB):
                    tsl = bass.ts(tb, 128)
                    for oc0, ocn in ((0, 512), (512, 256)):
                        pp = pjps.tile([128, 512], F32, tag="pp", name="pp")
                        for fb in range(3):
                            nc.tensor.matmul(
                                pp[:, 0:ocn],
                                lhsT=yT128[fb][:, tsl],
                                rhs=wp_sb[fb][:, oc0:oc0 + ocn],
                                start=(fb == 0),
                                stop=(fb == 2),
                            )
                        osb = pjout.tile([128, 512], F32, tag="osb", name="osb")
                        nc.scalar.copy(osb[:, 0:ocn], pp[:, 0:ocn])
                        nc.sync.dma_start(out=out[tsl, oc0:oc0 + ocn],
                                          in_=osb[:, 0:ocn])

    nc.finalize()
    return nc


def _bf16(a):
    return np.ascontiguousarray(np.asarray(a)).astype(ml_dtypes.bfloat16)


# permutation putting rope pairs into contiguous even/odd halves per head
_PERM64 = np.concatenate([np.arange(0, D, 2), np.arange(1, D, 2)])


def _prep_core(c, x, w_qkv, w_proj, sin_rep, cos_rep, tri_m, ident_m):
    b, hh = c // 2, c % 2
    wq = w_qkv[0 * C + hh * 384: 0 * C + hh * 384 + 384].reshape(HL, D, C)
    wk = w_qkv[1 * C + hh * 384: 1 * C + hh * 384 + 384].reshape(HL, D, C)
    wv = w_qkv[2 * C + hh * 384: 2 * C + hh * 384 + 384]
    wq = wq[:, _PERM64, :].reshape(HL * D, C)
    wk = wk[:, _PERM64, :].reshape(HL * D, C)
    w_local = np.concatenate([wq, wk, wv], 0)       # (1152, 768)
    return {
        "xT": _bf16(x[b].T),
        "wqkvT": _bf16(w_local.T),
        "wpT": _bf16(w_proj[:, hh * 384: hh * 384 + 384].T),
        "sinr": sin_rep,
        "cosr": cos_rep,
        "tri": tri_m,
        "ident": ident_m,
    }


def kernel(x, w_qkv, w_proj, rope_sin, rope_cos, _trace=False):
    global _CACHED_NC
    x = np.asarray(x, dtype=np.float32)
    w_qkv = np.asarray(w_qkv, dtype=np.float32)
    w_proj = np.asarray(w_proj, dtype=np.float32)
    rope_sin = np.asarray(rope_sin, dtype=np.float32)
    rope_cos = np.asarray(rope_cos, dtype=np.float32)

    # (T, 384): per head block [table(32) | table(32)]
    sin_rep = _bf16(np.tile(np.concatenate([rope_sin, rope_sin], 1), (1, HL)))
    cos_rep = _bf16(np.tile(np.concatenate([rope_cos, rope_cos], 1), (1, HL)))
    tri_m = _bf16(np.arange(128)[:, None] > np.arange(128)[None, :])
    negi_m = _bf16(np.eye(128) * -1e9)
    ident_m = _bf16(np.eye(128))

    in_maps = [_prep_core(c, x, w_qkv, w_proj, sin_rep, cos_rep, tri_m, ident_m)
               for c in range(NCORES)]
    for m in in_maps:
        m["negi"] = negi_m

    if _CACHED_NC is None:
        _CACHED_NC = build_nc()
    nc = _CACHED_NC

    res = run_bass_kernel_spmd(nc, in_maps, core_ids=list(range(NCORES)),
                               trace=_trace)
    parts = [res.results[c]["out"] for c in range(NCORES)]
    out = np.stack([parts[2 * b] + parts[2 * b + 1] for b in range(B)], 0)
    if _trace:
        return out.astype(np.float32), res
    return out.astype(np.float32)


# revision 21
# speedup vs baseline: 1.2687x; 1.0974x over previous
"""Causal self-attention with RoPE, sharded over 8 TRN2 NeuronCores.

Sharding: data-parallel over B (4 ways) x tensor-parallel over heads
(2 ways, 6 heads each). Each core computes qkv projection, RoPE,
causal attention and a partial output projection for its (batch,
head-half); the host sums the two head-half partials per batch.

Device matmuls run in bf16 (f32 PSUM accumulate). All attention
matmuls are uniform (K=128, M=128) shapes: kT is zero-padded per head
to a full 128-partition stationary, and V carries a ones column (for
the softmax denominator) plus zero padding to 128 columns. Softmax
needs no max-subtraction at these score magnitudes.

RoPE pairs are host-permuted into contiguous even/odd halves per head
(legal because the QK contraction is invariant to permuting head dims
as long as q and k share the permutation, and V is not roped).
"""

import numpy as np
import ml_dtypes

import concourse.bass as bass
import concourse.tile as tile
import concourse.mybir as mybir
from concourse import bacc
from concourse.bass_utils import run_bass_kernel_spmd

B, T, C, H, D = 4, 2048, 768, 12, 64
HL = H // 2          # heads per core
TB = T // 128        # 16 t-blocks
CB = C // 128        # 6 contraction blocks
NCORES = 8

F32 = mybir.dt.float32
BF16 = mybir.dt.bfloat16
AF = mybir.ActivationFunctionType

_CACHED_NC = None
DEBUG_DUMPS = False


def build_nc():
    nc = bacc.Bacc("TRN2", target_bir_lowering=False)

    xT = nc.declare_dram_parameter("xT", [C, T], BF16, isOutput=False)
    wqkvT = nc.declare_dram_parameter("wqkvT", [C, 3 * HL * D], BF16, isOutput=False)
    wpT = nc.declare_dram_parameter("wpT", [HL * D, C], BF16, isOutput=False)
    sinr = nc.declare_dram_parameter("sinr", [T, HL * D], BF16, isOutput=False)
    cosr = nc.declare_dram_parameter("cosr", [T, HL * D], BF16, isOutput=False)
    tri = nc.declare_dram_parameter("tri", [128, 128], BF16, isOutput=False)
    negi = nc.declare_dram_parameter("negi", [128, 128], BF16, isOutput=False)
    ident = nc.declare_dram_parameter("ident", [128, 128], BF16, isOutput=False)
    out = nc.declare_dram_parameter("out", [T, C], F32, isOutput=True)
    dbg = {}
    if DEBUG_DUMPS:
        dbg["dq"] = nc.declare_dram_parameter("dq", [128, 3 * T], BF16, isOutput=True)
        dbg["dk"] = nc.declare_dram_parameter("dk", [128, HL * T], BF16, isOutput=True)
        dbg["dv"] = nc.declare_dram_parameter("dv", [128, HL * 128], BF16, isOutput=True)
        dbg["dy"] = nc.declare_dram_parameter("dy", [64, T], BF16, isOutput=True)
        dbg["dden"] = nc.declare_dram_parameter("dden", [64, 4 * 512], F32, isOutput=True)

    with tile.TileContext(nc) as tc:
        with (
            tc.tile_pool(name="persist", bufs=1) as persist,
            tc.tile_pool(name="pPp", bufs=6) as pPp,
            tc.tile_pool(name="small", bufs=3) as small,
        ):
            # ---- persistent SBUF tensors ----
            # qT_all: head-pair block hb occupies cols [hb*2048, (hb+1)*2048);
            # head h lives at rows (h%2)*64 of block h//2.
            qT_all = persist.tile([128, 3 * T], BF16, tag="qT", name="qT_all")
            # kTp_all: head h occupies cols [h*2048, ...), rows (h%2)*64,
            # other 64 rows are zeros (full-K stationary for uniform matmuls).
            kTp_all = persist.tile([128, HL * T], BF16, tag="kT", name="kTp_all")
            # v tiles: per t-block, head h at cols [h*128, h*128+64] = v,
            # col h*128+64 = ones, cols h*128+65.. = zeros.
            v_sb = [persist.tile([128, HL, 128], BF16, tag=f"v{i}", name=f"v{i}")
                    for i in range(TB)]
            wp_sb = [persist.tile([128, C], BF16, tag=f"wp{i}", name=f"wp{i}")
                     for i in range(3)]
            yT64 = [persist.tile([64, T], BF16, tag=f"y64_{i}", name=f"y64_{i}")
                    for i in range(HL)]
            yT128 = [persist.tile([128, T], BF16, tag=f"y128_{i}", name=f"y128_{i}")
                     for i in range(3)]
            tri_sb = persist.tile([128, 128], BF16, tag="tri")
            negi_sb = persist.tile([128, 128], BF16, tag="negi")
            id_sb = persist.tile([128, 128], BF16, tag="ident")

            nc.gpsimd.dma_start(out=tri_sb, in_=tri[:, :])
            nc.gpsimd.dma_start(out=negi_sb, in_=negi[:, :])
            nc.gpsimd.dma_start(out=id_sb, in_=ident[:, :])
            for i in range(3):
                nc.gpsimd.dma_start(out=wp_sb[i], in_=wpT[i * 128:(i + 1) * 128, :])

            # zero the off-head rows of kTp_all once
            kTp_v = kTp_all.rearrange("p (c two t) -> p c two t", c=3, two=2)
            nc.vector.memset(kTp_v[0:64, :, 1:2, :], 0.0)
            nc.vector.memset(kTp_v[64:128, :, 0:1, :], 0.0)
            # zero/ones padding of v tiles once
            for i in range(TB):
                nc.vector.memset(v_sb[i][:, :, D:D + 1], 1.0)
                nc.vector.memset(v_sb[i][:, :, D + 1:128], 0.0)

            # ================= phase 1: qkv + rope + transpose =================
            with (
                tc.tile_pool(name="p1in", bufs=1) as p1in,
                tc.tile_pool(name="p1work", bufs=3) as p1w,
                tc.tile_pool(name="p1psum", bufs=3, space="PSUM") as p1ps,
                tc.tile_pool(name="ptpsum", bufs=2, space="PSUM") as ptps,
            ):
                xT_sb = [p1in.tile([128, T], BF16, tag=f"xT{i}", name=f"xTs{i}")
                         for i in range(CB)]
                w_sb = [p1in.tile([128, 3 * HL * D], BF16, tag=f"w{i}", name=f"ws{i}")
                        for i in range(CB)]
                for i in range(CB):
                    nc.sync.dma_start(out=xT_sb[i], in_=xT[i * 128:(i + 1) * 128, :])
                    nc.sync.dma_start(out=w_sb[i], in_=wqkvT[i * 128:(i + 1) * 128, :])

                for tb in range(TB):
                    tsl = bass.ts(tb, 128)
                    ps_qkv = []
                    for chunk in range(3):
                        ps = p1ps.tile([128, 384], F32, tag="qkv", name="psqkv")
                        for cb in range(CB):
                            nc.tensor.matmul(
                                ps,
                                lhsT=xT_sb[cb][:, tsl],
                                rhs=w_sb[cb][:, chunk * 384:(chunk + 1) * 384],
                                start=(cb == 0),
                                stop=(cb == CB - 1),
                            )
                        ps_qkv.append(ps)

                    sin_sb = p1w.tile([128, HL * D], BF16, tag="sin", name="sin_sb")
                    cos_sb = p1w.tile([128, HL * D], BF16, tag="cos", name="cos_sb")
                    nc.gpsimd.dma_start(out=sin_sb, in_=sinr[tsl, :])
                    nc.gpsimd.dma_start(out=cos_sb, in_=cosr[tsl, :])

                    # v: pack t-major (ones/zero padding already set)
                    nc.scalar.copy(
                        v_sb[tb][:, :, 0:D],
                        ps_qkv[2].rearrange("p (h d) -> p h d", h=HL),
                    )

                    # rope directly from PSUM; halves layout (host-permuted):
                    # per head cols [evens(32) | odds(32)]
                    for qk in range(2):
                        ro = p1w.tile([128, HL * D], BF16, tag=f"ro{qk}",
                                      name=f"ro{qk}")
                        t1 = p1w.tile([128, HL * D], BF16, tag="t1", name="t1")
                        t2 = p1w.tile([128, HL * D], BF16, tag="t2", name="t2")
                        nc.vector.tensor_mul(t1, ps_qkv[qk], cos_sb)
                        nc.vector.tensor_mul(t2, ps_qkv[qk], sin_sb)
                        rv = ro.rearrange("p (h half i) -> p h half i", h=HL, half=2)
                        t1v = t1.rearrange("p (h half i) -> p h half i", h=HL, half=2)
                        t2v = t2.rearrange("p (h half i) -> p h half i", h=HL, half=2)
                        nc.vector.tensor_sub(rv[:, :, 0:1, :], t1v[:, :, 0:1, :],
                                             t2v[:, :, 1:2, :])
                        nc.vector.tensor_add(rv[:, :, 1:2, :], t2v[:, :, 0:1, :],
                                             t1v[:, :, 1:2, :])

                        # transpose the 3 128-col blocks into one PSUM tile
                        pt = ptps.tile([128, 384], BF16, tag="pt", name="pt")
                        for cb2 in range(3):
                            nc.tensor.transpose(
                                pt[:, cb2 * 128:(cb2 + 1) * 128],
                                ro[:, cb2 * 128:(cb2 + 1) * 128], id_sb
                            )
                        if qk == 0:
                            qv = qT_all.rearrange("p (c t) -> p c t", c=3)
                            nc.scalar.copy(qv[:, :, tsl],
                                           pt.rearrange("p (c t) -> p c t", c=3))
                        else:
                            ptv = pt.rearrange("p (c t) -> p c t", c=3)
                            nc.scalar.copy(kTp_v[0:64, :, 0:1, tsl],
                                           ptv[0:64, :, :])
                            nc.scalar.copy(kTp_v[64:128, :, 1:2, tsl],
                                           ptv[64:128, :, :])

            # ================= phase 2: attention =================
            with (
                tc.tile_pool(name="psS", bufs=3, space="PSUM") as psS,
                tc.tile_pool(name="psO", bufs=2, space="PSUM") as psO,
            ):
                for h in range(HL):
                    hb = h // 2
                    for sq in range(4):
                        po = psO.tile([128, 512], F32, tag="po", name="po")
                        njs = 4 * sq + 4
                        jgroups = [list(range(g, min(g + 2, njs)))
                                   for g in range(0, njs, 2)]
                        for jg in jgroups:
                            ps3 = psS.tile([128, 1024], F32, tag="ps3", name="ps3")
                            pP3 = pPp.tile([128, 1024], BF16, tag="pP", name="pP")
                            offs = []
                            for slot, j in enumerate(jg):
                                diag_r = j - 4 * sq
                                col0 = 512 * sq + max(0, diag_r) * 128
                                N = 512 * (sq + 1) - col0
                                o = slot * 512
                                nc.tensor.matmul(
                                    ps3[:, o:o + N],
                                    lhsT=kTp_all[:, h * T + j * 128:
                                                 h * T + (j + 1) * 128],
                                    rhs=qT_all[:, hb * T + col0:hb * T + col0 + N],
                                    start=True, stop=(diag_r < 0),
                                )
                                if diag_r >= 0:
                                    # accumulate -1e9 on the upper triangle so
                                    # exp() zeroes the non-causal entries
                                    nc.tensor.matmul(
                                        ps3[:, o:o + 128],
                                        lhsT=negi_sb,
                                        rhs=tri_sb,
                                        start=False, stop=True,
                                        skip_group_check=True,
                                    )
                                offs.append((o, N, col0 - 512 * sq, diag_r))
                            otot = offs[-1][0] + offs[-1][1]
                            nc.scalar.activation(pP3[:, 0:otot], ps3[:, 0:otot],
                                                 AF.Exp, scale=0.125)
                            for (j, (o2, N, ls, diag_r)) in zip(jg, offs):
                                nc.tensor.matmul(
                                    po[:, ls:ls + N],
                                    lhsT=v_sb[j][:, h, :],
                                    rhs=pP3[:, o2:o2 + N],
                                    start=(j == 0),
                                    stop=(j == njs - 1),
                                )
                        posb = small.tile([D + 1, 512], F32, tag="posb", name="posb")
                        nc.vector.tensor_copy(out=posb, in_=po[0:D + 1, :])
                        rden = small.tile([1, 512], F32, tag="rden", name="rden")
                        nc.vector.tensor_copy(out=rden, in_=po[D:D + 1, :])
                        rec = small.tile([1, 512], F32, tag="rec", name="rec")
                        nc.vector.reciprocal_approx_fast(rec, rden)
                        bc = small.tile([64, 512], F32, tag="bc", name="bc")
                        nc.gpsimd.partition_broadcast(bc, rec)
                        if DEBUG_DUMPS and h == 0:
                            nc.sync.dma_start(out=dbg["dden"][:, sq * 512:(sq + 1) * 512], in_=bc)
                        nc.vector.tensor_mul(
                            yT64[h][:, bass.ts(sq, 512)], posb[0:D, :], bc
                        )
                    nc.sync.dma_start(
                        out=yT128[h // 2][(h % 2) * 64:(h % 2) * 64 + 64, :],
                        in_=yT64[h][:, :],
                    )

            if DEBUG_DUMPS:
                nc.sync.dma_start(out=dbg["dq"][:, :], in_=qT_all)
                nc.sync.dma_start(out=dbg["dk"][:, :], in_=kTp_all)
                nc.sync.dma_start(out=dbg["dv"][:, :], in_=v_sb[0].rearrange("p h d -> p (h d)"))
                nc.sync.dma_start(out=dbg["dy"][:, :], in_=yT64[0])

            # ================= phase 3: output projection (partial) ============
            with (
                tc.tile_pool(name="pjpsum", bufs=3, space="PSUM") as pjps,
                tc.tile_pool(name="pjout", bufs=3) as pjout,
            ):
                for tb in range(TB):
                    tsl = bass.ts(tb, 128)
                    for oc0, ocn in ((0, 512), (512, 256)):
                        pp = pjps.tile([128, 512], F32, tag="pp", name="pp")
                        for fb in range(3):
                            nc.tensor.matmul(
                                pp[:, 0:ocn],
                                lhsT=yT128[fb][:, tsl],
                                rhs=wp_sb[fb][:, oc0:oc0 + ocn],
                                start=(fb == 0),
                                stop=(fb == 2),
                            )
                        osb = pjout.tile([128, 512], F32, tag="osb", name="osb")
                        nc.scalar.copy(osb[:, 0:ocn], pp[:, 0:ocn])
                        nc.sync.dma_start(out=out[tsl, oc0:oc0 + ocn],
                                          in_=osb[:, 0:ocn])

    nc.finalize()
    return nc


def _bf16(a):
    return np.ascontiguousarray(np.asarray(a)).astype(ml_dtypes.bfloat16)


# permutation putting rope pairs into contiguous even/odd halves per head
_PERM64 = np.concatenate([np.arange(0, D, 2), np.arange(1, D, 2)])


def _prep_core(c, x, w_qkv, w_proj, sin_rep, cos_rep, tri_m, ident_m):
    b, hh = c // 2, c % 2
    wq = w_qkv[0 * C + hh * 384: 0 * C + hh * 384 + 384].reshape(HL, D, C)
    wk = w_qkv[1 * C + hh * 384: 1 * C + hh * 384 + 384].reshape(HL, D, C)
    wv = w_qkv[2 * C + hh * 384: 2 * C + hh * 384 + 384]
    wq = wq[:, _PERM64, :].reshape(HL * D, C)
    wk = wk[:, _PERM64, :].reshape(HL * D, C)
    w_local = np.concatenate([wq, wk, wv], 0)       # (1152, 768)
    return {
        "xT": _bf16(x[b].T),
        "wqkvT": _bf16(w_local.T),
        "wpT": _bf16(w_proj[:, hh * 384: hh * 384 + 384].T),
        "sinr": sin_rep,
        "cosr": cos_rep,
        "tri": tri_m,
        "ident": ident_m,
    }


def kernel(x, w_qkv, w_proj, rope_sin, rope_cos, _trace=False):
    global _CACHED_NC
    x = np.asarray(x, dtype=np.float32)
    w_qkv = np.asarray(w_qkv, dtype=np.float32)
    w_proj = np.asarray(w_proj, dtype=np.float32)
    rope_sin = np.asarray(rope_sin, dtype=np.float32)
    rope_cos = np.asarray(rope_cos, dtype=np.float32)

    # (T, 384): per head block [table(32) | table(32)]
    sin_rep = _bf16(np.tile(np.concatenate([rope_sin, rope_sin], 1), (1, HL)))
    cos_rep = _bf16(np.tile(np.concatenate([rope_cos, rope_cos], 1), (1, HL)))
    tri_m = _bf16(np.arange(128)[:, None] > np.arange(128)[None, :])
    negi_m = _bf16(np.eye(128) * -1e9)
    ident_m = _bf16(np.eye(128))

    in_maps = [_prep_core(c, x, w_qkv, w_proj, sin_rep, cos_rep, tri_m, ident_m)
               for c in range(NCORES)]
    for m in in_maps:
        m["negi"] = negi_m

    if _CACHED_NC is None:
        _CACHED_NC = build_nc()
    nc = _CACHED_NC

    res = run_bass_kernel_spmd(nc, in_maps, core_ids=list(range(NCORES)),
                               trace=_trace)
    parts = [res.results[c]["out"] for c in range(NCORES)]
    out = np.stack([parts[2 * b] + parts[2 * b + 1] for b in range(B)], 0)
    if _trace:
        return out.astype(np.float32), res
    return out.astype(np.float32)
